# revision 1
# baseline (speedup 1.0000x reference)
"""EnhancedGDN Trainium2 kernel.

Data-parallel over batch B=64 across 8 NeuronCores (8 graphs each).
All 64 graphs share one edge list -> host does integer-only index prep;
all float math runs on device.  363.8us (baseline) -> ~314-318us.

Design notes (what matters on this hardware):
  - ap_gather costs ~28ns PER INDEX in hidden gpsimd-sequencer stalls
    (the visible ~0.4us "APGather" slice is only the tail).  This was
    the baseline's largest hidden cost (145us).  Mitigations here:
      * only ONE gather family (si[dst] per out-CSR slot); softmax
        denominators come from the scattered dense W instead (DVE tile
        presum + one 1-row ones-matmul per half + reciprocal_approx_fast)
      * nodes are relabeled by out-degree (host perm) so per-src-chunk
        slot width K_j shrinks: 8 banded gather calls, num_idxs=16*K_j,
        total ~3.1k idx instead of 5.1k -> ~86us -> ~74us of stalls.
  - local_scatter zeroes its output by semantics; 4 merged calls/graph
    (num_elems=2000) build each graph's dense [1000,1000] f16 W.
  - x^T tiles (agg lhsT) are matmul'd directly from data (no explicit
    transposes); scores/emb-scores staged via [2,500] psum chunks.
  - wdata partition-spread DMAs hoisted out of the graph loop.
  - BN stats AllReduce is split: graphs 0-6 reduced under graph 7's
    compute; the tiny graph-7 AR remains (~27us exposed - protocol
    latency, not skew).  The fusion-MLP's t_out half is precomputed
    into SBUF during that window; the final f1 pass is then a single
    matmul + add + relu per chunk.
"""

import os

os.environ.setdefault("NEURON_RT_RESET_CORES", "1")

import numpy as np

import concourse.bass as bass
import concourse.bacc as bacc
import concourse.tile as tile
from concourse import mybir
from concourse.bass_utils import run_bass_kernel_spmd

B, N, D, E = 64, 1000, 128, 20000
M = 8          # devices
G = B // M     # graphs per device
NG = G * N     # nodes per device
NEG = 0.2
EPS = 1e-5

F16 = mybir.dt.float16
F32 = mybir.dt.float32
I16 = mybir.dt.int16
AF = mybir.ActivationFunctionType
ALU = mybir.AluOpType

_CACHE = {}


# ---------------------------------------------------------------- host index prep
def _wrap(flat_per_core):
    # flat_per_core: [8, L] -> wrapped [128, L//16] (ap_gather idx layout)
    L = flat_per_core.shape[1]
    w = np.zeros((128, L // 16), np.int16)
    for c in range(8):
        for p in range(16):
            w[16 * c + p] = flat_per_core[c, p::16]
    return w


def _prep_indices(edge_index):
    src = edge_index[0].astype(np.int64)
    dst = edge_index[1].astype(np.int64)
    key = dst * N + src
    uniq, cnt = np.unique(key, return_counts=True)
    ii = uniq // N          # dst
    jj = uniq % N           # src
    ii = np.concatenate([ii, np.arange(N)])   # self loops
    jj = np.concatenate([jj, np.arange(N)])
    cc = np.concatenate([cnt, np.ones(N, np.int64)]).astype(np.float32)

    # relabel nodes by out-degree desc -> per-chunk slot width K_j shrinks
    outdeg = np.bincount(jj, minlength=N)
    perm = np.argsort(-outdeg, kind="stable")     # new rank r -> old node
    rank = np.empty(N, np.int64)
    rank[perm] = np.arange(N)
    jj = rank[jj]
    ii = rank[ii]

    outdeg2 = np.bincount(jj, minlength=1024)
    Ks = tuple(max(4, (int(outdeg2[j * 128:(j + 1) * 128].max()) + 3) // 4 * 4)
               for j in range(8))
    SK = sum(Ks)
    koff = np.cumsum([0] + list(Ks))

    order = np.argsort(jj, kind="stable")
    oj, oi, oc = jj[order], ii[order], cc[order]
    starts = np.searchsorted(oj, np.arange(N))
    kpos = np.arange(len(oj)) - starts[oj]
    Kmax = max(Ks)
    gidx = np.full((1024, Kmax), 1000, np.int64)
    sidx = np.full((1024, Kmax), -1, np.int64)
    cnto = np.zeros((1024, Kmax), np.float32)
    gidx[oj, kpos] = oi
    sidx[oj, kpos] = oi
    cnto[oj, kpos] = oc

    # gather idx wrapped per call j (core c covers srcs 128j+16c+m);
    # cntAll in matching [row 16c+g, (call j, m, k)] layout
    gidxA = np.zeros((128, SK), np.int16)
    cntAll = np.zeros((128, 16 * SK), np.float16)
    off16 = 0
    for j, Kj in enumerate(Ks):
        for c in range(8):
            lst = gidx[128 * j + 16 * c:128 * j + 16 * c + 16, :Kj].reshape(-1)
            for p in range(16):
                gidxA[16 * c + p, koff[j]:koff[j] + Kj] = lst[p::16]
            cblk = cnto[128 * j + 16 * c:128 * j + 16 * c + 16, :Kj].reshape(-1)
            for g in range(8):
                cntAll[16 * c + g, off16:off16 + 16 * Kj] = cblk
        off16 += 16 * Kj

    # merged-pair scatter idx: tiles (2q, 2q+1) -> num_elems=2000 per call
    scat2 = np.full((128, SK), -1, np.int64)
    for t in range(8):
        Kt = Ks[t]
        blk = sidx[t * 128:(t + 1) * 128, :Kt]
        off = 1000 if (t % 2 == 1) else 0
        scat2[:, koff[t]:koff[t] + Kt] = np.where(blk >= 0, blk + off, -1)
    scat2 = scat2.astype(np.int16)

    return dict(Ks=Ks, SK=SK, perm=perm, gidxA=gidxA, cntAll=cntAll,
                scat2=scat2)


# ---------------------------------------------------------------- device module
def _build(Ks, n_cores, dbg=False):
    SK = sum(Ks)        # total per-src slot columns (banded)
    NOUT = 16 * SK      # slots per gather-layout partition row
    koff = [0]
    for k_ in Ks:
        koff.append(koff[-1] + k_)
    off16 = [16 * o for o in koff]
    NH = 16             # 500-wide chunks over 8000
    CH = 500

    nc = bacc.Bacc("TRN2", target_bir_lowering=False, debug=False,
                   num_devices=n_cores)

    def din(name, shape, dt):
        return nc.dram_tensor(name, shape, dt, kind="ExternalInput").ap()

    x0T = din("x0T", [128, NG], F16)
    embT = din("embT", [128, N], F16)
    wpack = din("wpack", [128, 902], F16)
    bpack = din("bpack", [128, 8], F32)
    outb = din("outb", [1, 1], F32)
    gidxA_d = din("gidxA", [128, SK], I16)
    cnt_d = din("cntAll", [128, NOUT], F16)
    scat2_d = din("scat2", [128, SK], I16)
    y_out = nc.dram_tensor("y", [1, NG], F32, kind="ExternalOutput").ap()

    if dbg:
        def dout(name, shape, dt):
            return nc.dram_tensor(name, shape, dt, kind="ExternalOutput").ap()
        d_sNN = dout("d_sNN", [16, 1024], F32)
        d_cw = dout("d_cw", [128, NOUT], F16)
        d_agg = dout("d_agg", [128, NG], F16)
        d_wt = dout("d_wt", [128, 8000], F16)

    cc_in = nc.dram_tensor("cc_in", [128, 2], F32).ap()
    cc_out = nc.dram_tensor("cc_out", [128, 2], F32, addr_space="Shared").ap()
    cc_b_in = nc.dram_tensor("cc_b_in", [128, 2], F32).ap()
    cc_b_out = nc.dram_tensor("cc_b_out", [128, 2], F32, addr_space="Shared").ap()
    cc_win = nc.dram_tensor("cc_win", [128, 2], F32).ap()
    cc_wout = nc.dram_tensor("cc_wout", [128, 2], F32, addr_space="Shared").ap()

    # wpack column layout
    W_LINT, W_V, W_F1, W_ATTQ, W_ATTEM, W_OUT, W_ONER, W_F2P, W_LINP, W_ONE = (
        0, 128, 256, 512, 514, 516, 517, 645, 773, 901)
    # bpack columns
    B_VB, B_GNN, B_FB1, B_FB2, B_GAM, B_BET, B_EPS = 0, 1, 2, 3, 4, 5, 6

    with tile.TileContext(nc) as tc:
        with (
            tc.tile_pool(name="cst", bufs=1) as cst,
            tc.tile_pool(name="big", bufs=1) as big,
            tc.tile_pool(name="wt", bufs=3) as wtp,
            tc.tile_pool(name="sm", bufs=1) as sm,
            tc.tile_pool(name="stg", bufs=8) as stg,
            tc.tile_pool(name="rdp", bufs=2) as rdp,
            tc.tile_pool(name="psA", bufs=4, space="PSUM") as psA,
            tc.tile_pool(name="psS", bufs=4, space="PSUM") as psS,
        ):
            wp = cst.tile([128, 902], F16)
            nc.sync.dma_start(wp[:], wpack)
            x0 = big.tile([128, NG], F16, tag="x0")
            for q in range(16):
                nc.sync.dma_start(x0[:, q * 500:(q + 1) * 500],
                                  x0T[:, q * 500:(q + 1) * 500])
            bp = cst.tile([128, 8], F32)
            nc.sync.dma_start(bp[:], bpack)
            ob = cst.tile([1, 1], F32)
            nc.sync.dma_start(ob[:], outb)
            emb = cst.tile([128, N], F16)
            nc.sync.dma_start(emb[:], embT)
            gxA = cst.tile([128, SK], I16)
            nc.sync.dma_start(gxA[:], gidxA_d)
            ocnt = big.tile([128, NOUT], F16, tag="cnt")
            nc.sync.dma_start(ocnt[:], cnt_d)
            sct = cst.tile([128, SK], I16)
            nc.sync.dma_start(sct[:], scat2_d)

            def bias(col):
                return bp[:, col:col + 1]

            # warm up the collective path early (absorbs setup/skew)
            warm = sm.tile([128, 2], F32)
            nc.vector.memset(warm[:], 0.0)
            nc.sync.dma_start(cc_win, warm[:])
            nc.gpsimd.collective_compute(
                "AllReduce", ALU.add,
                replica_groups=[list(range(n_cores))],
                ins=[cc_win], outs=[cc_wout])

            # ---- node scores -> sNN [16, 1024]: rows 0-7 si[g], 8-15 sj[g]
            sNN = sm.tile([16, 1024], F32)
            nc.vector.memset(sNN[:], 0.0)
            attc = sm.tile([128, 2], F16)
            pat = psA.tile([128, CH], F32, tag="A")
            nc.tensor.matmul(pat[:, 0:2], wp[:, W_LINP:W_LINP + 128],
                             wp[:, W_ATTQ:W_ATTQ + 2], start=True, stop=True)
            nc.vector.tensor_copy(attc[:], pat[:, 0:2])
            for h in range(NH):
                s = h * CH
                g, off = divmod(s, 1000)
                ps = psS.tile([2, CH], F32, tag="S")
                nc.tensor.matmul(ps[:], attc[:, 0:2],
                                 x0[:, s:s + CH], start=True, stop=False)
                nc.tensor.matmul(ps[:], wp[:, W_ATTEM:W_ATTEM + 2],
                                 emb[:, off:off + CH], start=False, stop=True)
                st = stg.tile([2, CH], F32, tag="sc")
                nc.vector.tensor_copy(st[:], ps[:])
                nc.sync.dma_start(
                    sNN[:, off:off + CH].rearrange("(a g) f -> g a f", a=2)[g],
                    st[:, :])
            if dbg:
                nc.sync.dma_start(d_sNN, sNN[:])

            # ---- broadcast tables
            # T16 rows 16c+g: si[g] (A-gather table); rows 16c+8+g: sj[g] (B)
            # -> rows 16c..16c+15 are exactly sNN: one copy per core
            T16 = big.tile([128, 1024], F32, tag="T16")
            nc.vector.memset(T16[:], 0.0)
            for c in range(8):
                nc.sync.dma_start(T16[16 * c:16 * c + 16, :], sNN[:])
            # bcC row 16c+g col 16j+m: sj[g][128j + 16c + m] (banded layout)
            bcC = sm.tile([128, 128], F32)
            nc.vector.memset(bcC[:], 0.0)
            for g in range(G):
                for c in range(8):
                    nc.sync.dma_start(
                        bcC[16 * c + g:16 * c + g + 1, :].rearrange(
                            "p (j m) -> p j m", j=8),
                        sNN[8 + g:9 + g, 0:1024].rearrange(
                            "p (j c m) -> c p j m", c=8, m=16)[c])

            # ---- gather + edge chain (out-CSR rows 16c+g meaningful)
            # cw = cnt*exp(lrelu(si[dst]+sj[src])) -> scatter source
            g1 = big.tile([128, NOUT], F32, tag="g1")
            cw = big.tile([128, NOUT], F16, tag="cw")
            for j in range(8):
                Kj = Ks[j]
                sl = slice(off16[j], off16[j] + 16 * Kj)
                bcb = bcC[:, 16 * j:16 * j + 16].unsqueeze(2).broadcast_to(
                    [128, 16, Kj])
                nc.gpsimd.ap_gather(g1[:, sl], T16[:],
                                    gxA[:, koff[j]:koff[j] + Kj], channels=128,
                                    num_elems=1024, d=1, num_idxs=16 * Kj)
                nc.vector.tensor_tensor(
                    cw[:, sl].rearrange("p (n k) -> p n k", k=Kj),
                    g1[:, sl].rearrange("p (n k) -> p n k", k=Kj),
                    bcb, op=ALU.add)
                nc.vector.scalar_tensor_tensor(cw[:, sl], cw[:, sl], NEG, cw[:, sl],
                                               op0=ALU.mult, op1=ALU.max)
                nc.scalar.activation(cw[:, sl], cw[:, sl], AF.Exp)
                nc.vector.tensor_tensor(cw[:, sl], cw[:, sl], ocnt[:, sl],
                                        op=ALU.mult)
            if dbg:
                nc.sync.dma_start(d_cw, cw[:])

            # ---- wdata spread: cw rows -> scatter layout (hoisted, round-robin)
            wdAll = big.tile([128, 8 * SK], F16, tag="wd")
            for g in range(G):
                for j in range(8):
                    Kj = Ks[j]
                    nc.sync.dma_start(
                        wdAll[:, g * SK + koff[j]:g * SK + koff[j] + Kj],
                        cw[g::16, off16[j]:off16[j] + 16 * Kj].rearrange(
                            "p (m k) -> p m k", k=Kj))

            # ---- xnm: x^T tiles direct from data (lhsT for agg matmuls)
            # xnm[p, (g*8+t)*128 + c] = x[g*1000 + t*128 + p, c]
            xnm = big.tile([128, 64 * 128], F16, tag="xnm")
            for g in range(G):
                for tq in range(2):     # 4 tiles per psum
                    px = psA.tile([128, 512], F32, tag="A")
                    for j in range(4):
                        t = tq * 4 + j
                        s = g * 1000 + t * 128
                        w = 128 if t < 7 else 104
                        nc.tensor.matmul(px[0:w, j * 128:(j + 1) * 128],
                                         x0[:, s:s + w],
                                         wp[:, W_LINT:W_LINT + 128],
                                         start=True, stop=True)
                    dst = xnm[:, (g * 8 + tq * 4) * 128:
                              (g * 8 + tq * 4 + 4) * 128]
                    if (g * 2 + tq) % 2 == 0:
                        nc.scalar.activation(dst, px[:], AF.Identity)
                    else:
                        nc.vector.tensor_copy(dst, px[:])

            # ---- tT = v_w @ x0T + v_b (needed only at fusion MLP)
            tT = big.tile([128, NG], F16, tag="tT")
            for h in range(NH):
                s = h * CH
                ps2 = psA.tile([128, CH], F32, tag="A")
                nc.tensor.matmul(ps2[:], wp[:, W_V:W_V + 128],
                                 x0[:, s:s + CH], start=True, stop=True)
                if h % 2 == 0:
                    nc.scalar.activation(tT[:, s:s + CH], ps2[:], AF.Identity,
                                         bias=bias(B_VB))
                else:
                    nc.vector.tensor_scalar(tT[:, s:s + CH], ps2[:], bias(B_VB),
                                            None, op0=ALU.add)

            # composite head: cvec = f_w2 @ out_w ; cb = <out_w, f_b2> + out_b
            cvec = sm.tile([128, 2], F16)
            cb = sm.tile([1, 2], F32)
            nc.vector.tensor_copy(cvec[:, 1:2], bias(B_FB2))
            pc = psS.tile([2, 1], F32, tag="S")
            nc.tensor.matmul(pc[0:1, 0:1], cvec[:, 1:2],
                             wp[:, W_OUT:W_OUT + 1], start=True, stop=True)
            pc2 = psA.tile([128, CH], F32, tag="A")
            nc.tensor.matmul(pc2[:, 0:1], wp[:, W_F2P:W_F2P + 128],
                             wp[:, W_OUT:W_OUT + 1], start=True, stop=True)
            nc.vector.tensor_copy(cvec[:, 0:1], pc2[:, 0:1])
            nc.vector.tensor_copy(cb[:, 0:1], pc[0:1, 0:1])
            nc.vector.tensor_tensor(cb[:, 1:2], cb[:, 0:1], ob[:], op=ALU.add)
            # ---- graph loop: scatter W, agg matmul, normalize, BN accum
            aggT = big.tile([128, NG], F16, tag="g1")   # alias: g1 dead
            scr = big.tile([128, 1000], F16, tag="cnt")  # alias: ocnt dead
            sumacc = sm.tile([128, 8], F32)
            sqacc = sm.tile([128, 8], F32)
            for g in range(G):
                WT = wtp.tile([128, 8000], F16, tag="wt")
                for q in range(4):
                    nc.gpsimd.local_scatter(
                        WT[:, q * 2000:(q + 1) * 2000],
                        wdAll[:, g * SK + koff[2 * q]:g * SK + koff[2 * q + 2]],
                        sct[:, koff[2 * q]:koff[2 * q + 2]],
                        channels=128, num_elems=2000,
                        num_idxs=koff[2 * q + 2] - koff[2 * q])
                if dbg and g == G - 1:
                    nc.sync.dma_start(d_wt, WT[:])
                # den[d] = sum_s W[s, d]: presum the 8 src tiles on DVE, then
                # one 1-row ones-matmul per half; fast reciprocal; broadcast.
                wsum = rdp.tile([128, 1000], F16, tag="ws")
                nc.vector.tensor_tensor(wsum[:], WT[:, 0:1000],
                                        WT[:, 1000:2000], op=ALU.add)
                for t in range(2, 8):
                    nc.vector.tensor_tensor(wsum[:], wsum[:],
                                            WT[:, t * 1000:(t + 1) * 1000],
                                            op=ALU.add)
                den2 = rdp.tile([1, 1000], F32, tag="dn")
                for hf in range(2):
                    pd = psS.tile([2, CH], F32, tag="S")
                    nc.tensor.matmul(pd[0:1, :], wp[:, W_ONE:W_ONE + 1],
                                     wsum[:, hf * CH:(hf + 1) * CH],
                                     start=True, stop=True)
                    nc.vector.tensor_copy(den2[0:1, hf * CH:(hf + 1) * CH],
                                          pd[0:1, :])
                nc.vector.reciprocal_approx_fast(den2[:], den2[:])
                rdg = rdp.tile([1, 1000], F16, tag="rdg")
                nc.vector.tensor_copy(rdg[:], den2[:])
                rdf = rdp.tile([128, 1000], F16, tag="rdf")
                for hf in range(2):
                    pr = psA.tile([128, CH], F32, tag="A")
                    nc.tensor.matmul(pr[:], wp[0:1, W_ONER:W_ONER + 128],
                                     rdg[0:1, hf * CH:hf * CH + CH],
                                     start=True, stop=True)
                    nc.vector.tensor_copy(rdf[:, hf * CH:(hf + 1) * CH], pr[:])
                for hf in range(2):
                    pa = psA.tile([128, CH], F32, tag="A")
                    for t in range(8):
                        kt = 128 if t < 7 else 104
                        nc.tensor.matmul(
                            pa[:], xnm[0:kt, (g * 8 + t) * 128:
                                       (g * 8 + t) * 128 + 128],
                            WT[0:kt, t * 1000 + hf * CH:t * 1000 + hf * CH + CH],
                            start=(t == 0), stop=(t == 7))
                    nc.vector.scalar_tensor_tensor(
                        aggT[:, g * 1000 + hf * CH:g * 1000 + hf * CH + CH],
                        pa[:], 1.0, rdf[:, hf * CH:(hf + 1) * CH],
                        op0=ALU.mult, op1=ALU.mult)
                # per-graph BN partial sums on ACT
                nc.scalar.activation(scr[:, 0:1000],
                                     aggT[:, g * 1000:(g + 1) * 1000],
                                     AF.Identity, accum_out=sumacc[:, g:g + 1])
                nc.scalar.activation(scr[:, 0:1000],
                                     aggT[:, g * 1000:(g + 1) * 1000],
                                     AF.Square, accum_out=sqacc[:, g:g + 1])
                if g == 6:
                    # split-AR part A: raw sums of graphs 0..6, overlapped
                    # under graph 7's scatter+matmul work
                    statsA = sm.tile([128, 2], F32)
                    nc.vector.tensor_reduce(statsA[:, 0:1], sumacc[:, 0:7],
                                            axis=mybir.AxisListType.X, op=ALU.add)
                    nc.vector.tensor_reduce(statsA[:, 1:2], sqacc[:, 0:7],
                                            axis=mybir.AxisListType.X, op=ALU.add)
                    nc.sync.dma_start(cc_in, statsA[:])
                    nc.gpsimd.collective_compute(
                        "AllReduce", ALU.add,
                        replica_groups=[list(range(n_cores))],
                        ins=[cc_in], outs=[cc_out])
            if dbg:
                nc.sync.dma_start(d_agg, aggT[:])

            # ---- split-AR part B: graph 7 only; combine + fold gnn_bias
            statsB = sm.tile([128, 2], F32)
            nc.vector.tensor_copy(statsB[:, 0:1], sumacc[:, 7:8])
            nc.vector.tensor_copy(statsB[:, 1:2], sqacc[:, 7:8])
            nc.sync.dma_start(cc_b_in, statsB[:])
            nc.gpsimd.collective_compute(
                "AllReduce", ALU.add,
                replica_groups=[list(range(n_cores))],
                ins=[cc_b_in], outs=[cc_b_out])
            # f1 tT-half precomputed while the AllReduce is in flight
            ht = big.tile([128, NG], F16, tag="wd")   # alias: wdAll dead
            for h in range(NH):
                s = h * CH
                ph = psA.tile([128, CH], F32, tag="A")
                nc.tensor.matmul(ph[:], wp[:, W_F1 + 128:W_F1 + 256],
                                 tT[:, s:s + CH], start=True, stop=True)
                nc.scalar.activation(ht[:, s:s + CH], ph[:], AF.Identity,
                                     bias=bias(B_FB1))
            gsa = sm.tile([128, 2], F32)
            nc.sync.dma_start(gsa[:], cc_out)
            gsb = sm.tile([128, 2], F32)
            nc.sync.dma_start(gsb[:], cc_b_out)
            graw = sm.tile([128, 2], F32)
            nc.vector.tensor_tensor(graw[:], gsa[:], gsb[:], op=ALU.add)
            gstats = sm.tile([128, 2], F32)
            s1u = sm.tile([128, 4], F32)
            gb = bias(B_GNN)
            nc.vector.tensor_scalar(s1u[:, 2:3], gb, float(B * N), None,
                                    op0=ALU.mult)
            nc.vector.tensor_tensor(gstats[:, 0:1], graw[:, 0:1], s1u[:, 2:3],
                                    op=ALU.add)
            nc.vector.scalar_tensor_tensor(gstats[:, 1:2], graw[:, 0:1], 2.0,
                                           s1u[:, 2:3], op0=ALU.mult, op1=ALU.add)
            nc.vector.tensor_tensor(gstats[:, 1:2], gstats[:, 1:2], gb,
                                    op=ALU.mult)
            nc.vector.tensor_tensor(gstats[:, 1:2], gstats[:, 1:2], graw[:, 1:2],
                                    op=ALU.add)

            # ---- BN coefficients A, Bv
            cf = sm.tile([128, 8], F32)
            mu, msq, var, rsd, A_, Bv = (cf[:, i:i + 1] for i in range(6))
            inv_n = 1.0 / (B * N)
            nc.vector.tensor_scalar_mul(mu, gstats[:, 0:1], inv_n)
            nc.vector.tensor_scalar_mul(msq, gstats[:, 1:2], inv_n)
            nc.vector.tensor_tensor(var, mu, mu, op=ALU.mult)
            nc.vector.tensor_sub(var, msq, var)
            nc.scalar.activation(var, var, AF.Sqrt, bias=bias(B_EPS))
            nc.vector.reciprocal(rsd, var)
            nc.vector.tensor_tensor(A_, bias(B_GAM), rsd, op=ALU.mult)
            nc.vector.tensor_tensor(Bv, mu, A_, op=ALU.mult)
            nc.vector.tensor_sub(Bv, bias(B_BET), Bv)
            nc.vector.tensor_tensor(cf[:, 6:7], bias(B_GNN), A_, op=ALU.mult)
            nc.vector.tensor_tensor(Bv, Bv, cf[:, 6:7], op=ALU.add)

            # ---- fused tail: BN-apply + f1 + head, chunk-pipelined
            hT = big.tile([128, NG], F16, tag="x0")   # alias: x0 is dead
            for h in range(NH):
                s = h * CH
                if h % 2 == 0:
                    nc.scalar.activation(aggT[:, s:s + CH], aggT[:, s:s + CH],
                                         AF.Relu, bias=Bv, scale=A_)
                else:
                    nc.vector.tensor_scalar(aggT[:, s:s + CH], aggT[:, s:s + CH],
                                            A_, Bv, op0=ALU.mult, op1=ALU.add)
                    nc.vector.tensor_scalar_max(aggT[:, s:s + CH],
                                                aggT[:, s:s + CH], 0.0)
                ps = psA.tile([128, CH], F32, tag="A")
                nc.tensor.matmul(ps[:], wp[:, W_F1:W_F1 + 128],
                                 aggT[:, s:s + CH], start=True, stop=True)
                nc.vector.tensor_tensor(hT[:, s:s + CH], ps[:], ht[:, s:s + CH],
                                        op=ALU.add)
                if h % 2 == 0:
                    nc.vector.tensor_scalar_max(hT[:, s:s + CH], hT[:, s:s + CH],
                                                0.0)
                else:
                    nc.scalar.activation(hT[:, s:s + CH], hT[:, s:s + CH], AF.Relu)
                ps2 = psS.tile([2, CH], F32, tag="S")
                nc.tensor.matmul(ps2[0:1, :], cvec[:, 0:1],
                                 hT[:, s:s + CH], start=True, stop=True)
                yst = stg.tile([2, CH], F32, tag="sc")
                nc.vector.tensor_scalar(yst[0:1, :], ps2[0:1, :], cb[:, 1:2],
                                        None, op0=ALU.add)
                nc.sync.dma_start(y_out[:, s:s + CH], yst[0:1, :])

    nc.compile()
    return nc


# ---------------------------------------------------------------- entry point
def _prepare(inputs, dbg=False):
    data = np.asarray(inputs["data"], np.float32)
    edge_index = np.asarray(inputs["edge_index"])

    pre = _prep_indices(edge_index)
    perm = pre["perm"]

    key = (pre["Ks"], dbg)
    if key not in _CACHE:
        _CACHE[key] = _build(pre["Ks"], M, dbg=dbg)
    nc = _CACHE[key]

    f16 = np.float16

    def t16(a):
        return np.ascontiguousarray(np.asarray(a, np.float32).T).astype(f16)

    wpack = np.zeros((128, 902), f16)
    wpack[:, 0:128] = t16(inputs["lin_w"])
    wpack[:, 128:256] = t16(inputs["v_w"])
    wpack[:, 256:512] = np.ascontiguousarray(
        np.asarray(inputs["f_w1"], np.float32).T).astype(f16).reshape(2, 128, 128
        ).transpose(1, 0, 2).reshape(128, 256)
    wpack[:, 512] = np.asarray(inputs["att_i"], np.float32).astype(f16)
    wpack[:, 513] = np.asarray(inputs["att_j"], np.float32).astype(f16)
    wpack[:, 514] = np.asarray(inputs["att_em_i"], np.float32).astype(f16)
    wpack[:, 515] = np.asarray(inputs["att_em_j"], np.float32).astype(f16)
    wpack[:, 516] = np.asarray(inputs["out_w"], np.float32)[0].astype(f16)
    wpack[0, 517:645] = 1.0
    wpack[:, 645:773] = np.asarray(inputs["f_w2"], np.float32).astype(f16)
    wpack[:, 773:901] = np.asarray(inputs["lin_w"], np.float32).astype(f16)
    wpack[:, 901] = 1.0

    bpack = np.zeros((128, 8), np.float32)
    bpack[:, 0] = np.asarray(inputs["v_b"], np.float32)
    bpack[:, 1] = np.asarray(inputs["gnn_bias"], np.float32)
    bpack[:, 2] = np.asarray(inputs["f_b1"], np.float32)
    bpack[:, 3] = np.asarray(inputs["f_b2"], np.float32)
    bpack[:, 4] = np.asarray(inputs["bn_gamma"], np.float32)
    bpack[:, 5] = np.asarray(inputs["bn_beta"], np.float32)
    bpack[:, 6] = EPS
    outb = np.asarray(inputs["out_b"], np.float32).reshape(1, 1)

    embT = t16(np.asarray(inputs["emb"], np.float32)[perm])

    shared = dict(
        embT=embT, wpack=wpack, bpack=bpack, outb=outb,
        gidxA=pre["gidxA"], cntAll=pre["cntAll"],
        scat2=pre["scat2"],
    )
    in_maps = []
    dataP = data[:, perm, :]
    for d in range(M):
        x0Tn = np.ascontiguousarray(
            dataP[d * G:(d + 1) * G].transpose(2, 0, 1).reshape(128, NG)
        ).astype(f16)
        in_maps.append(dict(shared, x0T=x0Tn))
    return nc, in_maps, perm


def kernel(**inputs):
    nc, in_maps, perm = _prepare(inputs)
    res = run_bass_kernel_spmd(nc, in_maps, list(range(M)))
    out = np.empty(B * N, np.float32)
    for d in range(M):
        blk = res.results[d]["y"].reshape(G, N)
        ub = np.empty((G, N), np.float32)
        ub[:, perm] = blk
        out[d * NG:(d + 1) * NG] = ub.reshape(-1)
    return out



# revision 4
# speedup vs baseline: 1.3091x; 1.3091x over previous
"""EnhancedGDN Trainium2 kernel (dense factorized edge-softmax rewrite).

Data-parallel over batch B=64 across 8 NeuronCores (8 graphs each).

Key identity: exp(leaky_relu(si+sj, 0.2)) = max(exp(si+sj), exp(0.2si+0.2sj))
— both branches are rank-1 over (src, dst), so the edge weights become
  W[s,d] = C[s,d] * max(Ei[d]Ej[s], Fi[d]Fj[s])
with C the (host-built, graph-independent) edge-count mask including self
loops.  This removes every gather/scatter/index table from the old design:
  - per graph: 16 ACT Exp passes (bias = transposed sj scores, per-partition),
    DVE max + mask multiply, PE ones-matmul denominators, PE agg matmuls,
    fused normalize+BN-partial STTs with accum_out.
  - scores si/sj come from one [2,500]-psum matmul chain; sj is transposed
    to per-partition columns with PE is_transpose matmuls (identity rhs).
  - temporal path folded on host: ht = (f_w1[:,D:]@v_w) @ x + (f_w1[:,D:]@v_b
    + f_b1); head folded to cvec = f_w2.T@out_w, cb = out_w@f_b2 + out_b.
  - single stats AllReduce; ht precompute fills its latency.
"""

import os

os.environ.setdefault("NEURON_RT_RESET_CORES", "1")

import numpy as np

import concourse.bass as bass
import concourse.bacc as bacc
import concourse.tile as tile
from concourse import mybir
from concourse.bass_utils import run_bass_kernel_spmd

B, N, D, E = 64, 1000, 128, 20000
M = 8          # devices
G = B // M     # graphs per device
NG = G * N     # nodes per device
NEG = 0.2
EPS = 1e-5

F16 = mybir.dt.float16
F32 = mybir.dt.float32
AF = mybir.ActivationFunctionType
ALU = mybir.AluOpType

# wpack columns
W_LINT, W_HT, W_F1A, W_ATTC, W_ONE, W_CVEC, W_EYE = (
    0, 128, 256, 384, 386, 387, 388)
WP_COLS = 396
# bpack columns
B_HT, B_GNN, B_GAM, B_BET, B_EPS, B_CB = 0, 1, 2, 3, 4, 5

_CACHE = {}


def _build(n_cores):
    nc = bacc.Bacc("TRN2", target_bir_lowering=False, debug=False,
                   num_devices=n_cores)

    def din(name, shape, dt):
        return nc.dram_tensor(name, shape, dt, kind="ExternalInput").ap()

    x0T = din("x0T", [128, NG], F16)
    cmask = din("cmask", [128, 8000], F16)
    wpack = din("wpack", [128, WP_COLS], F16)
    bpack = din("bpack", [128, 8], F32)
    embsc = din("embsc", [2, 1024], F32)
    y_out = nc.dram_tensor("y", [1, NG], F32, kind="ExternalOutput").ap()

    cc_in = nc.dram_tensor("cc_in", [128, 2], F32).ap()
    cc_out = nc.dram_tensor("cc_out", [128, 2], F32, addr_space="Shared").ap()
    cc_win = nc.dram_tensor("cc_win", [128, 2], F32).ap()
    cc_wout = nc.dram_tensor("cc_wout", [128, 2], F32, addr_space="Shared").ap()

    with tile.TileContext(nc) as tc:
        with (
            tc.tile_pool(name="cst", bufs=1) as cst,
            tc.tile_pool(name="big", bufs=1) as big,
            tc.tile_pool(name="wt", bufs=2) as wtp,
            tc.tile_pool(name="vt", bufs=2) as vtp,
            tc.tile_pool(name="sib", bufs=2) as sibp,
            tc.tile_pool(name="rdp", bufs=2) as rdp,
            tc.tile_pool(name="sm", bufs=1) as sm,
            tc.tile_pool(name="stg", bufs=4) as stg,
            tc.tile_pool(name="psA", bufs=3, space="PSUM") as psA,
            tc.tile_pool(name="psS", bufs=4, space="PSUM") as psS,
            tc.tile_pool(name="psT", bufs=1, space="PSUM") as psT,
        ):
            wp = cst.tile([128, WP_COLS], F16)
            nc.sync.dma_start(wp[:], wpack)
            bp = cst.tile([128, 8], F32)
            nc.sync.dma_start(bp[:], bpack)
            emc = cst.tile([2, 1024], F32)
            nc.sync.dma_start(emc[:], embsc)
            x0 = big.tile([128, NG], F16, tag="x0")
            for g in range(G):
                nc.sync.dma_start(x0[:, g * 1000:(g + 1) * 1000],
                                  x0T[:, g * 1000:(g + 1) * 1000])
            C = big.tile([128, 8000], F16, tag="C")
            for q in range(4):
                nc.sync.dma_start(C[:, q * 2000:(q + 1) * 2000],
                                  cmask[:, q * 2000:(q + 1) * 2000])

            def bias(col):
                return bp[:, col:col + 1]

            # warm up the collective path early (absorbs setup skew)
            warm = sm.tile([128, 2], F32)
            nc.vector.memset(warm[:], 0.0)
            nc.sync.dma_start(cc_win, warm[:])
            nc.gpsimd.collective_compute(
                "AllReduce", ALU.add,
                replica_groups=[list(range(n_cores))],
                ins=[cc_win], outs=[cc_wout])

            # ---- scores: si -> siA row 0 (per-graph 1024 slices),
            #              sj -> sjA rows g (for PE transposes)
            siA = sm.tile([1, 8192], F16)
            sjA = sm.tile([8, 1024], F16)
            nc.vector.memset(sjA[:], 0.0)
            for g in range(G):
                st = stg.tile([2, 1000], F16, tag="sc")
                for hf in range(2):
                    ps = psS.tile([2, 500], F32, tag="S")
                    nc.tensor.matmul(ps[:], wp[:, W_ATTC:W_ATTC + 2],
                                     x0[:, g * 1000 + hf * 500:
                                        g * 1000 + hf * 500 + 500],
                                     start=True, stop=True)
                    nc.vector.scalar_tensor_tensor(
                        st[:, hf * 500:hf * 500 + 500], ps[:], 1.0,
                        emc[:, hf * 500:hf * 500 + 500],
                        op0=ALU.mult, op1=ALU.add)
                nc.sync.dma_start(siA[0:1, g * 1024:g * 1024 + 1000],
                                  st[0:1, :])
                nc.sync.dma_start(sjA[g:g + 1, 0:1000], st[1:2, :])

            # ---- sj transposes -> sjT columns [p, j*8+g]
            ptT = psT.tile([128, 64], F16, tag="T")
            for j in range(8):
                nc.tensor.matmul(ptT[:, j * 8:(j + 1) * 8],
                                 sjA[0:8, j * 128:(j + 1) * 128],
                                 wp[0:8, W_EYE:W_EYE + 8], is_transpose=True)
            sjTE = sm.tile([128, 64], F32)
            nc.vector.tensor_copy(sjTE[:], ptT[:])
            sjTF = sm.tile([128, 64], F32)
            nc.vector.tensor_scalar_mul(sjTF[:], sjTE[:], NEG)

            # ---- xnm: x^T tiles direct from data (lhsT for agg matmuls)
            # xnm[p, (g*8+t)*128 + c] = x[g*1000 + t*128 + p, c]
            xnm = big.tile([128, 64 * 128], F16, tag="xnm")
            for g in range(G):
                for tq in range(2):
                    px = psA.tile([128, 512], F32, tag="A")
                    for j in range(4):
                        t = tq * 4 + j
                        s = g * 1000 + t * 128
                        w = 128 if t < 7 else 104
                        nc.tensor.matmul(px[0:w, j * 128:(j + 1) * 128],
                                         x0[:, s:s + w],
                                         wp[:, W_LINT:W_LINT + 128],
                                         start=True, stop=True)
                    dst = xnm[:, (g * 8 + tq * 4) * 128:
                              (g * 8 + tq * 4 + 4) * 128]
                    if tq % 2 == 0:
                        nc.scalar.activation(dst, px[:], AF.Identity)
                    else:
                        nc.vector.tensor_copy(dst, px[:])

            # ---- graph loop
            aggT = big.tile([128, NG], F16, tag="agg")
            sqscr = sm.tile([128, 512], F16)
            sumacc = sm.tile([128, 16], F32)
            sqacc = sm.tile([128, 16], F32)
            for g in range(G):
                Sib = sibp.tile([128, 1024], F16, tag="sib")
                nc.gpsimd.partition_broadcast(
                    Sib[:, 0:1000], siA[0:1, g * 1024:g * 1024 + 1000])
                Wt = wtp.tile([128, 8000], F16, tag="wt")
                Vt = vtp.tile([128, 8000], F16, tag="vt")
                for t in range(8):
                    nc.scalar.activation(Wt[:, t * 1000:(t + 1) * 1000],
                                         Sib[:, 0:1000], AF.Exp,
                                         bias=sjTE[:, t * 8 + g:t * 8 + g + 1])
                for t in range(8):
                    nc.scalar.activation(Vt[:, t * 1000:(t + 1) * 1000],
                                         Sib[:, 0:1000], AF.Exp,
                                         bias=sjTF[:, t * 8 + g:t * 8 + g + 1],
                                         scale=NEG)
                nc.vector.tensor_tensor(Wt[:], Wt[:], Vt[:], op=ALU.max)
                nc.vector.tensor_tensor(Wt[:], Wt[:], C[:], op=ALU.mult)

                # denominators: ones-matmul column sums, fast reciprocal
                den2 = rdp.tile([1, 1024], F32, tag="dn")
                for hf in range(2):
                    pd = psS.tile([1, 500], F32, tag="S")
                    for t in range(8):
                        nc.tensor.matmul(
                            pd[:], wp[:, W_ONE:W_ONE + 1],
                            Wt[:, t * 1000 + hf * 500:t * 1000 + hf * 500 + 500],
                            start=(t == 0), stop=(t == 7))
                    nc.vector.tensor_copy(den2[0:1, hf * 500:hf * 500 + 500],
                                          pd[:])
                nc.vector.reciprocal_approx_fast(den2[0:1, 0:1000],
                                                 den2[0:1, 0:1000])
                rdg = rdp.tile([1, 1024], F16, tag="rdg")
                nc.vector.tensor_copy(rdg[0:1, 0:1000], den2[0:1, 0:1000])
                rdf = rdp.tile([128, 1024], F16, tag="rdf")
                nc.gpsimd.partition_broadcast(rdf[:, 0:1000], rdg[0:1, 0:1000])

                # agg matmuls + fused normalize / BN partial accumulation
                for hf in range(2):
                    pa = psA.tile([128, 512], F32, tag="A")
                    for t in range(8):
                        kt = 128 if t < 7 else 104
                        nc.tensor.matmul(
                            pa[:, 0:500], xnm[0:kt, (g * 8 + t) * 128:
                                              (g * 8 + t) * 128 + 128],
                            Wt[0:kt, t * 1000 + hf * 500:t * 1000 + hf * 500 + 500],
                            start=(t == 0), stop=(t == 7))
                    sl = slice(g * 1000 + hf * 500, g * 1000 + hf * 500 + 500)
                    nc.vector.scalar_tensor_tensor(
                        aggT[:, sl], pa[:, 0:500], 1.0, rdf[:, hf * 500:hf * 500 + 500],
                        op0=ALU.mult, op1=ALU.mult,
                        accum_out=sumacc[:, 2 * g + hf:2 * g + hf + 1])
                    nc.vector.scalar_tensor_tensor(
                        sqscr[:, 0:500], aggT[:, sl], 1.0, aggT[:, sl],
                        op0=ALU.mult, op1=ALU.mult,
                        accum_out=sqacc[:, 2 * g + hf:2 * g + hf + 1])

            # ---- single stats AllReduce
            statsA = sm.tile([128, 2], F32)
            nc.vector.tensor_reduce(statsA[:, 0:1], sumacc[:],
                                    axis=mybir.AxisListType.X, op=ALU.add)
            nc.vector.tensor_reduce(statsA[:, 1:2], sqacc[:],
                                    axis=mybir.AxisListType.X, op=ALU.add)
            nc.sync.dma_start(cc_in, statsA[:])
            nc.gpsimd.collective_compute(
                "AllReduce", ALU.add,
                replica_groups=[list(range(n_cores))],
                ins=[cc_in], outs=[cc_out])

            # ht (temporal half) precomputed while the AllReduce is in flight
            ht = big.tile([128, NG], F16, tag="ht")
            for h in range(16):
                s = h * 500
                ph = psA.tile([128, 512], F32, tag="A")
                nc.tensor.matmul(ph[:, 0:500], wp[:, W_HT:W_HT + 128],
                                 x0[:, s:s + 500], start=True, stop=True)
                nc.scalar.activation(ht[:, s:s + 500], ph[:, 0:500],
                                     AF.Identity, bias=bias(B_HT))

            graw = sm.tile([128, 2], F32)
            nc.sync.dma_start(graw[:], cc_out)
            # fold gnn_bias into stats: sum += b*BN ; sumsq += 2b*sum + b^2*BN
            gstats = sm.tile([128, 2], F32)
            s1u = sm.tile([128, 4], F32)
            gb = bias(B_GNN)
            nc.vector.tensor_scalar(s1u[:, 2:3], gb, float(B * N), None,
                                    op0=ALU.mult)
            nc.vector.tensor_tensor(gstats[:, 0:1], graw[:, 0:1], s1u[:, 2:3],
                                    op=ALU.add)
            nc.vector.scalar_tensor_tensor(gstats[:, 1:2], graw[:, 0:1], 2.0,
                                           s1u[:, 2:3], op0=ALU.mult, op1=ALU.add)
            nc.vector.tensor_tensor(gstats[:, 1:2], gstats[:, 1:2], gb,
                                    op=ALU.mult)
            nc.vector.tensor_tensor(gstats[:, 1:2], gstats[:, 1:2], graw[:, 1:2],
                                    op=ALU.add)

            # BN coefficients A_, Bv  (s_out = relu(A_*agg + Bv), agg pre-bias)
            cf = sm.tile([128, 8], F32)
            mu, msq, var, rsd, A_, Bv = (cf[:, i:i + 1] for i in range(6))
            inv_n = 1.0 / (B * N)
            nc.vector.tensor_scalar_mul(mu, gstats[:, 0:1], inv_n)
            nc.vector.tensor_scalar_mul(msq, gstats[:, 1:2], inv_n)
            nc.vector.tensor_tensor(var, mu, mu, op=ALU.mult)
            nc.vector.tensor_sub(var, msq, var)
            nc.scalar.activation(var, var, AF.Sqrt, bias=bias(B_EPS))
            nc.vector.reciprocal(rsd, var)
            nc.vector.tensor_tensor(A_, bias(B_GAM), rsd, op=ALU.mult)
            nc.vector.tensor_tensor(Bv, mu, A_, op=ALU.mult)
            nc.vector.tensor_sub(Bv, bias(B_BET), Bv)
            nc.vector.tensor_tensor(cf[:, 6:7], bias(B_GNN), A_, op=ALU.mult)
            nc.vector.tensor_tensor(Bv, Bv, cf[:, 6:7], op=ALU.add)

            # ---- fused tail: BN-apply + f1 + head, chunk-pipelined
            hT = big.tile([128, NG], F16, tag="C")   # alias: C is dead
            for h in range(16):
                s = h * 500
                if h % 2 == 0:
                    nc.scalar.activation(aggT[:, s:s + 500], aggT[:, s:s + 500],
                                         AF.Relu, bias=Bv, scale=A_)
                else:
                    nc.vector.tensor_scalar(aggT[:, s:s + 500], aggT[:, s:s + 500],
                                            A_, Bv, op0=ALU.mult, op1=ALU.add)
                    nc.vector.tensor_scalar_max(aggT[:, s:s + 500],
                                                aggT[:, s:s + 500], 0.0)
                pf = psA.tile([128, 512], F32, tag="A")
                nc.tensor.matmul(pf[:, 0:500], wp[:, W_F1A:W_F1A + 128],
                                 aggT[:, s:s + 500], start=True, stop=True)
                nc.vector.tensor_tensor(hT[:, s:s + 500], pf[:, 0:500],
                                        ht[:, s:s + 500], op=ALU.add)
                if h % 2 == 0:
                    nc.vector.tensor_scalar_max(hT[:, s:s + 500],
                                                hT[:, s:s + 500], 0.0)
                else:
                    nc.scalar.activation(hT[:, s:s + 500], hT[:, s:s + 500],
                                         AF.Relu)
                ph2 = psS.tile([2, 500], F32, tag="S")
                nc.tensor.matmul(ph2[0:1, :], wp[:, W_CVEC:W_CVEC + 1],
                                 hT[:, s:s + 500], start=True, stop=True)
                yst = stg.tile([1, 512], F32, tag="y32")
                nc.vector.tensor_scalar(yst[0:1, 0:500], ph2[0:1, :],
                                        bp[0:1, B_CB:B_CB + 1], None,
                                        op0=ALU.add)
                nc.sync.dma_start(y_out[:, s:s + 500], yst[0:1, 0:500])

    nc.compile()
    return nc


# ---------------------------------------------------------------- host prep
def _prep_cmask(edge_index):
    src = edge_index[0].astype(np.int64)
    dst = edge_index[1].astype(np.int64)
    loop = np.arange(N, dtype=np.int64)
    src = np.concatenate([src, loop])
    dst = np.concatenate([dst, loop])
    cm = np.zeros((128, 8000), np.float32)
    t = src // 128
    p = src % 128
    np.add.at(cm, (p, t * 1000 + dst), 1.0)
    return cm.astype(np.float16)


def _prepare(inputs):
    data = np.asarray(inputs["data"], np.float32)
    edge_index = np.asarray(inputs["edge_index"])

    if "nc" not in _CACHE:
        _CACHE["nc"] = _build(M)
    nc = _CACHE["nc"]

    f16 = np.float16
    lin_w = np.asarray(inputs["lin_w"], np.float32)
    v_w = np.asarray(inputs["v_w"], np.float32)
    f_w1 = np.asarray(inputs["f_w1"], np.float32)
    f_w2 = np.asarray(inputs["f_w2"], np.float32)
    out_w = np.asarray(inputs["out_w"], np.float32)
    att_i = np.asarray(inputs["att_i"], np.float32)
    att_j = np.asarray(inputs["att_j"], np.float32)
    att_em_i = np.asarray(inputs["att_em_i"], np.float32)
    att_em_j = np.asarray(inputs["att_em_j"], np.float32)
    emb = np.asarray(inputs["emb"], np.float32)
    v_b = np.asarray(inputs["v_b"], np.float32)
    f_b1 = np.asarray(inputs["f_b1"], np.float32)
    f_b2 = np.asarray(inputs["f_b2"], np.float32)
    out_b = np.asarray(inputs["out_b"], np.float32)

    f1a = f_w1[:, :D]                     # s_out half
    f1b = f_w1[:, D:]                     # t_out half
    ht_w = f1b @ v_w                      # [D, D]
    b_ht = f1b @ v_b + f_b1               # [D]
    cvec = f_w2.T @ out_w[0]              # [D]
    cb = float(out_w[0] @ f_b2 + out_b[0])

    wpack = np.zeros((128, WP_COLS), f16)
    wpack[:, W_LINT:W_LINT + 128] = np.ascontiguousarray(lin_w.T).astype(f16)
    wpack[:, W_HT:W_HT + 128] = np.ascontiguousarray(ht_w.T).astype(f16)
    wpack[:, W_F1A:W_F1A + 128] = np.ascontiguousarray(f1a.T).astype(f16)
    wpack[:, W_ATTC] = (lin_w.T @ att_i).astype(f16)
    wpack[:, W_ATTC + 1] = (lin_w.T @ att_j).astype(f16)
    wpack[:, W_ONE] = 1.0
    wpack[:, W_CVEC] = cvec.astype(f16)
    wpack[0:8, W_EYE:W_EYE + 8] = np.eye(8, dtype=f16)

    bpack = np.zeros((128, 8), np.float32)
    bpack[:, B_HT] = b_ht
    bpack[:, B_GNN] = np.asarray(inputs["gnn_bias"], np.float32)
    bpack[:, B_GAM] = np.asarray(inputs["bn_gamma"], np.float32)
    bpack[:, B_BET] = np.asarray(inputs["bn_beta"], np.float32)
    bpack[:, B_EPS] = EPS
    bpack[:, B_CB] = cb

    embsc = np.zeros((2, 1024), np.float32)
    embsc[0, :N] = emb @ att_em_i
    embsc[1, :N] = emb @ att_em_j

    cm = _prep_cmask(edge_index)

    shared = dict(cmask=cm, wpack=wpack, bpack=bpack, embsc=embsc)
    in_maps = []
    for d in range(M):
        x0Tn = np.ascontiguousarray(
            data[d * G:(d + 1) * G].transpose(2, 0, 1).reshape(128, NG)
        ).astype(f16)
        in_maps.append(dict(shared, x0T=x0Tn))
    return nc, in_maps, None


def kernel(**inputs):
    nc, in_maps, _ = _prepare(inputs)
    res = run_bass_kernel_spmd(nc, in_maps, list(range(M)))
    out = np.empty(B * N, np.float32)
    for d in range(M):
        out[d * NG:(d + 1) * NG] = res.results[d]["y"].reshape(-1)
    return out
